# revision 5
# baseline (speedup 1.0000x reference)
"""Distributed 2-layer GAT kernel for 8 Trainium2 NeuronCores — v2.

Same host-side graph preprocessing as v1 (degree-sorted relabeling, dst-major
edge slots, SPMD per-core programs keyed by the compile-time degree schedule
ghat).  Device-side restructured around the measured bottlenecks:

  * dma_gather descriptor prep on the Q7 (~8ns/row) is the critical resource:
    gathers are issued as prepare_only preps with a PA-deep software pipeline
    so prep for chunk i+PA overlaps trigger+DMA+compute for chunk i, and the
    first preps run under the phase-0/AllGather window.
  * AllGather outputs use addr_space="Shared" (single shared table instead of
    8 replicated copies) when USE_SHARED_AG is set.
  * Per-block epilogues are batched in groups of 5 blocks (one AG2 chunk) and
    ordered op-type-major so the scalar engine's activation table is loaded
    once per function per group instead of once per block.
  * LN uses Rsqrt (one ACT op) and bf16 intermediates.
  * Phase-2 groups run heaviest-first so the last AG2 chunk covers the
    lightest blocks (short tail before phase 4).
"""
import sys

sys.path.insert(0, "/opt/trn_rl_repo")

import numpy as np
import ml_dtypes

from concourse import bass, bacc, tile, mybir
from concourse import bass_utils
from concourse.masks import make_identity

BF16 = ml_dtypes.bfloat16
F32 = mybir.dt.float32
BF = mybir.dt.bfloat16
I16 = mybir.dt.int16
AF = mybir.ActivationFunctionType
OP = mybir.AluOpType

# problem constants
N, E = 20000, 320000
D_IN, HID, D_OUT = 128, 128, 32
H1, H2 = 4, 1
EPS = 1e-5

NCORES = 8
P = 128
NPAD = 20480
NBLK_G = NPAD // P
NPB = NPAD // NCORES    # 2560
NBLK = NPB // P         # 20 blocks per core
NEG = -1e9

T1COLS = 640            # L1 table row (bf16): 512 feats | 4 f32 a_src | pad
T2COLS = 256            # L2 table row (bf16): 128 feats | 1 f32 a_src | pad
KC = 16                 # max in-edge slots per gather call
NAG = 4                 # AllGather chunks
GRP = NBLK // NAG       # blocks per epilogue/AG group (5)
NSWQ = 4                # SWDGE queues
PA = 3                  # gather prep-ahead depth (gp bufs = PA + 1)

USE_SHARED_AG = False
SINGLE_PACKET = False     # True crashes NRT with 1280B rows (packet limit)
SP_BY_LAYER = {1: False, 2: False}  # single_packet=True crashes NRT (both 1280B and 512B rows)
USE_PREP = False

CC_GIN, CC_BIN = 0, 128
CC_G1, CC_B1, CC_BIAS1 = 256, 768, 1280
CC_G2, CC_B2, CC_BIAS2 = 1792, 1920, 2048
CC_BO = 2176
NCC = 2208


def _tid(n):
    blk = n // P
    c = blk % NCORES
    l = blk // NCORES
    cb = NBLK // NAG
    return ((l // cb) * (NPAD // NAG) + c * (NPB // NAG)
            + (l % cb) * P + n % P)


def prepare_inputs(x, edge_index):
    x = np.asarray(x, dtype=np.float32)
    ei = np.asarray(edge_index)
    src = np.concatenate([ei[0], np.arange(N, dtype=ei.dtype)]).astype(np.int64)
    dst = np.concatenate([ei[1], np.arange(N, dtype=ei.dtype)]).astype(np.int64)

    deg = np.bincount(dst, minlength=N)
    order = np.argsort(deg, kind="stable")
    newid = np.empty(N, dtype=np.int64)
    newid[order] = np.arange(N) + (NPAD - N)

    degp = np.zeros(NPAD, dtype=np.int64)
    degp[newid] = deg
    gmax = degp.reshape(NBLK_G, P).max(axis=1)
    ghat = gmax.reshape(NBLK, NCORES).max(axis=1)
    S = int(P * ghat.sum())

    nd = newid[dst]
    csr_order = np.argsort(nd, kind="stable")
    nsrc_sorted = newid[src[csr_order]]
    indptr = np.zeros(NPAD + 1, dtype=np.int64)
    np.cumsum(np.bincount(nd, minlength=NPAD), out=indptr[1:])

    tid_of = _tid(np.arange(NPAD))

    goff = np.zeros(NBLK, dtype=np.int64)
    goff[1:] = np.cumsum(ghat)[:-1]

    idxw = np.zeros((NCORES, P, S // 16), dtype=np.int16)
    x_own = np.zeros((NCORES, NPB, D_IN), dtype=np.float32)
    alsfix = np.zeros((NCORES, NPB, 8), dtype=np.float32)

    inv_new = np.full(NPAD, -1, dtype=np.int64)
    inv_new[newid] = np.arange(N)

    for c in range(NCORES):
        gblk = np.arange(NBLK) * NCORES + c
        nid = (gblk[:, None] * P + np.arange(P)).reshape(-1)
        ov = inv_new[nid]
        real = ov >= 0
        x_own[c][real] = x[ov[real]]
        alsfix[c][~real, :] = NEG

        idx_flat = np.zeros(S, dtype=np.int16)
        for l in range(NBLK):
            d0 = nid[l * P:(l + 1) * P]
            base = goff[l] * P
            for p in range(P):
                d = d0[p]
                s0, s1 = indptr[d], indptr[d + 1]
                ks = np.arange(s1 - s0)
                idx_flat[base + ks * P + p] = tid_of[nsrc_sorted[s0:s1]]
        idxw[c] = np.tile(idx_flat.reshape(S // 16, 16).T, (NCORES, 1))

    return {
        "ghat": [int(g) for g in ghat],
        "S": S,
        "idxw": idxw,
        "x_own": x_own,
        "alsfix": alsfix,
        "newid": newid,
    }


def prepare_weights(W1, att1_s, att1_d, bias1, g1, b1, g_in, b_in,
                    W2, att2_s, att2_d, bias2, g2, b2, Wo, bo):
    W1 = np.asarray(W1, np.float32)
    W2 = np.asarray(W2, np.float32)
    w1ext = np.zeros((D_IN, 520), dtype=BF16)
    w1ext[:, :512] = W1
    W1h = W1.reshape(D_IN, H1, HID)
    w1ext[:, 512:516] = np.einsum("khc,hc->kh", W1h, np.asarray(att1_s, np.float32))
    w1ext[:, 516:520] = np.einsum("khc,hc->kh", W1h, np.asarray(att1_d, np.float32))

    w2e = np.zeros((4 * HID, 130), dtype=np.float32)
    w2e[:, :128] = W2
    w2e[:, 128] = W2 @ np.asarray(att2_s, np.float32)[0]
    w2e[:, 129] = W2 @ np.asarray(att2_d, np.float32)[0]
    w2ext = np.ascontiguousarray(
        w2e.reshape(4, P, 130).transpose(1, 0, 2)).astype(BF16)

    woext = np.asarray(Wo, np.float32).astype(BF16)

    cc = np.zeros(NCC, dtype=np.float32)
    cc[CC_GIN:CC_GIN + 128] = g_in
    cc[CC_BIN:CC_BIN + 128] = b_in
    cc[CC_G1:CC_G1 + 512] = g1
    cc[CC_B1:CC_B1 + 512] = b1
    cc[CC_BIAS1:CC_BIAS1 + 512] = bias1
    cc[CC_G2:CC_G2 + 128] = g2
    cc[CC_B2:CC_B2 + 128] = b2
    cc[CC_BIAS2:CC_BIAS2 + 128] = bias2
    cc[CC_BO:CC_BO + 32] = bo
    colconst = np.tile(cc[None, :], (P, 1))

    return {"w1ext": w1ext, "w2ext": w2ext.reshape(P, 4 * 130),
            "woext": woext, "colconst": colconst}


def _bap(ap, dims):
    return bass.AP(ap.tensor, ap.offset, [ap.ap[0]] + [list(d) for d in dims])


def build_program(ghat, num_devices=NCORES, sim_safe=False):
    S = int(P * sum(ghat))
    goff = np.zeros(NBLK, dtype=np.int64)
    goff[1:] = np.cumsum(ghat)[:-1]

    nc = bacc.Bacc("TRN2", target_bir_lowering=False, debug=False,
                   num_devices=num_devices, num_swdge_queues=NSWQ)

    x_own = nc.dram_tensor("x_own", [NPB, D_IN], F32, kind="ExternalInput")
    idxw = nc.dram_tensor("idxw", [P, S // 16], I16, kind="ExternalInput")
    alsfix = nc.dram_tensor("alsfix", [NPB, 8], F32, kind="ExternalInput")
    w1ext = nc.dram_tensor("w1ext", [D_IN, 520], BF, kind="ExternalInput")
    w2ext = nc.dram_tensor("w2ext", [P, 4 * 130], BF, kind="ExternalInput")
    woext = nc.dram_tensor("woext", [P, D_OUT], BF, kind="ExternalInput")
    colconst = nc.dram_tensor("colconst", [P, NCC], BF, kind="ExternalInput")
    out = nc.dram_tensor("out", [NPB, D_OUT], F32, kind="ExternalOutput")

    if USE_SHARED_AG:
        ag1_out_t = nc.dram_tensor("ag1o", [NPAD, T1COLS], BF,
                                   kind="Internal", addr_space="Shared")
        ag2_out_t = nc.dram_tensor("ag2o", [NPAD, T2COLS], BF,
                                   kind="Internal", addr_space="Shared")

    rg = [list(range(num_devices))]

    # processing order for phase 2: heaviest group first
    grp_w = [sum(ghat[g * GRP:(g + 1) * GRP]) for g in range(NAG)]
    grp_order2 = sorted(range(NAG), key=lambda g: -grp_w[g])

    with tile.TileContext(nc) as tc:
        with (
            tc.tile_pool(name="cst", bufs=1) as cst,
            tc.tile_pool(name="wp", bufs=2) as wp,
            tc.tile_pool(name="hp", bufs=1) as hp,
            tc.tile_pool(name="wq", bufs=2) as wq,
            tc.tile_pool(name="gp", bufs=PA + 1) as gp,
            tc.tile_pool(name="ps", bufs=2, space="PSUM") as ps,
            tc.tile_pool(name="pss", bufs=2, space="PSUM") as pss,
            tc.tile_pool(name="dram", bufs=1, space="DRAM") as dram,
        ):
            # ---- constants (idx first: unblocks gather preps) ----
            idx_sb = cst.tile([P, S // 16], I16)
            nc.sync.dma_start(idx_sb[:], idxw[:])
            ident = cst.tile([P, P], BF)
            make_identity(nc, ident[:])
            w1s = cst.tile([P, 520], BF)
            nc.sync.dma_start(w1s[:], w1ext[:])
            w2s = cst.tile([P, 4, 130], BF)
            nc.sync.dma_start(w2s[:], w2ext[:])
            wos = cst.tile([P, D_OUT], BF)
            nc.sync.dma_start(wos[:], woext[:])
            ccb = cst.tile([P, NCC], BF)
            nc.sync.dma_start(ccb[:], colconst[:])
            afix = cst.tile([P, NBLK, 8], F32)
            nc.sync.dma_start(
                afix[:], bass.AP(alsfix.ap().tensor, 0,
                                 [[8, P], [8 * P, NBLK], [1, 8]]))
            eps_t = cst.tile([P, 1], F32)
            nc.vector.memset(eps_t[:], EPS)

            ald1 = cst.tile([P, NBLK, H1], F32)
            ald2 = cst.tile([P, NBLK, 1], F32)

            ag1_in = dram.tile([NPB, T1COLS], BF)
            ag2_in = dram.tile([NPB, T2COLS], BF)
            if USE_SHARED_AG:
                ag1_out = ag1_out_t
                ag2_out = ag2_out_t
            else:
                ag1_out = dram.tile([NPAD, T1COLS], BF)
                ag2_out = dram.tile([NPAD, T2COLS], BF)

            dsem = [nc.alloc_semaphore(f"gdma{q}") for q in range(NSWQ)]

            def transpose_to(dst_bf, src_bf):
                pst = pss.tile([P, P], BF, tag="tp")
                nc.tensor.transpose(out=pst[:], in_=src_bf, identity=ident[:])
                nc.vector.tensor_copy(out=dst_bf, in_=pst[:])

            # ---------- phase 0: LN0 + W1 matmul, build L1 table shard ----
            def phase0():
                for t in range(NBLK):
                    xt = wp.tile([P, D_IN], F32, tag="xt")
                    nc.sync.dma_start(xt[:], x_own[t * P:(t + 1) * P, :])
                    # LN in f32 input precision, bf16 out
                    mu = wp.tile([P, 1], F32, tag="p0mu")
                    nc.vector.tensor_reduce(out=mu[:], in_=xt[:],
                                            axis=mybir.AxisListType.X, op=OP.add)
                    nc.scalar.mul(mu[:], mu[:], 1.0 / D_IN)
                    xc = wp.tile([P, D_IN], F32, tag="p0xc")
                    nc.vector.tensor_scalar_sub(out=xc[:], in0=xt[:], scalar1=mu[:])
                    sq = wp.tile([P, D_IN], F32, tag="p0sq")
                    ss = wp.tile([P, 1], F32, tag="p0ss")
                    nc.scalar.activation(sq[:], xc[:], AF.Square, accum_out=ss[:])
                    sd = wp.tile([P, 1], F32, tag="p0sd")
                    nc.scalar.activation(sd[:], ss[:], AF.Sqrt,
                                         bias=eps_t[:], scale=1.0 / D_IN)
                    rstd = wp.tile([P, 1], F32, tag="p0rs")
                    nc.vector.reciprocal(rstd[:], sd[:])
                    nc.vector.tensor_scalar_mul(out=xc[:], in0=xc[:], scalar1=rstd[:])
                    nc.vector.tensor_mul(out=xc[:], in0=xc[:],
                                         in1=ccb[:, CC_GIN:CC_GIN + 128])
                    xnb = wp.tile([P, D_IN], BF, tag="xnb")
                    nc.vector.tensor_tensor(out=xnb[:], in0=xc[:],
                                            in1=ccb[:, CC_BIN:CC_BIN + 128],
                                            op=OP.add)
                    xT = wp.tile([P, P], BF, tag="xT")
                    transpose_to(xT[:], xnb[:])
                    ps1 = ps.tile([P, 512], F32, tag="big")
                    nc.tensor.matmul(ps1[:], lhsT=xT[:], rhs=w1s[:, 0:512],
                                     start=True, stop=True)
                    ps2_t = pss.tile([P, 130], F32, tag="mm2")
                    ps2 = ps2_t[:, 0:8]
                    nc.tensor.matmul(ps2[:], lhsT=xT[:], rhs=w1s[:, 512:520],
                                     start=True, stop=True)
                    tt = wp.tile([P, T1COLS], BF, tag="tt")
                    nc.vector.memset(tt[:, 520:T1COLS], 0)
                    nc.vector.tensor_copy(out=tt[:, 0:512], in_=ps1[:])
                    nc.vector.tensor_tensor(
                        out=tt[:, 512:520].bitcast(F32), in0=ps2[:, 0:4],
                        in1=afix[:, t, 0:4], op=OP.add)
                    nc.vector.tensor_copy(out=ald1[:, t, :], in_=ps2[:, 4:8])
                    nc.sync.dma_start(ag1_in[t * P:(t + 1) * P, :], tt[:])
                    if t % GRP == GRP - 1:
                        j = t // GRP
                        r0, r1 = j * NPB // NAG, (j + 1) * NPB // NAG
                        nc.gpsimd.collective_compute(
                            "AllGather", OP.bypass, replica_groups=rg,
                            ins=[ag1_in[r0:r1, :].opt()],
                            outs=[ag1_out[j * NPAD // NAG:
                                          (j + 1) * NPAD // NAG, :].opt()])

            # ---------- gather pipeline machinery ----------
            jobs = []   # list of (prep_fn, compute_fn)
            qrr = [0]

            def run_jobs(head=0):
                n = len(jobs)
                if not USE_PREP:
                    for i in range(n):
                        if i >= head:
                            jobs[i][0]()
                        jobs[i][1]()
                else:
                    for i in range(n + PA):
                        if head <= i < n:
                            jobs[i][0]()
                        j = i - PA
                        if 0 <= j < n:
                            jobs[j][1]()
                jobs.clear()

            # ---------- phase 2: GAT layer 1 ----------
            def gat_layer(layer, blocks_by_group, tcols, heads, epilogue):
                """Queue gather+aggregate jobs for `blocks_by_group` and
                attach the batched epilogue after each group."""
                tag = f"g{layer}"
                table = (ag1_out[0:NPAD, :] if layer == 1
                         else ag2_out[0:NPAD, :])
                aldt = ald1 if layer == 1 else ald2
                state = {}

                for gi, grp in enumerate(blocks_by_group):
                    for bi, l in enumerate(grp):
                        g = ghat[l]
                        chunks = []
                        k0 = 0
                        while k0 < g:
                            kn = min(KC, g - k0)
                            chunks.append((k0, kn))
                            k0 += kn
                        for ci, (k0, kn) in enumerate(chunks):
                            last_of_grp = (bi == len(grp) - 1
                                           and ci == len(chunks) - 1)
                            jobs.append(_make_job(
                                layer, tag, table, aldt, l, k0, kn,
                                ci == 0, k0 + kn >= g, tcols, heads, state,
                                epilogue if last_of_grp else None, grp))

            def _make_job(layer, tag, table, aldt, l, k0, kn, first, last,
                          tcols, heads, state, epi, grp):
                gt = gp.tile([P, KC, tcols], BF, tag=tag)
                q = qrr[0] % NSWQ
                qrr[0] += 1

                def prep():
                    if USE_PREP:
                        nc.gpsimd.dma_gather(
                            gt[:, 0:kn, :], table,
                            idx_sb[:, 8 * (int(goff[l]) + k0):
                                   8 * (int(goff[l]) + k0 + kn)],
                            P * kn, P * kn, tcols, prepare_only=True,
                            sem=dsem[q], single_packet=SP_BY_LAYER[layer],
                            queue_num=q)
                    else:
                        nc.gpsimd.dma_gather(
                            gt[:, 0:kn, :], table,
                            idx_sb[:, 8 * (int(goff[l]) + k0):
                                   8 * (int(goff[l]) + k0 + kn)],
                            P * kn, P * kn, tcols,
                            single_packet=SP_BY_LAYER[layer], queue_num=q)

                def compute():
                    if USE_PREP:
                        nc.gpsimd.trigger_dma(count=None, queue_num=q)
                    C = 512 if layer == 1 else 128
                    fb = 512 if layer == 1 else 128   # feature bytes offset
                    if first:
                        psA_t = ps.tile([P, 512], F32, tag="big", name=f"psA{layer}_{l}")
                        den_t = wp.tile([P, heads], F32, tag=f"den{layer}",
                                        name=f"den{layer}_{l}")
                        state["psA"] = psA_t
                        state["den"] = den_t
                    psA = state["psA"][:, 0:C]
                    den = state["den"]
                    als_v = gt[:, 0:kn, fb:fb + 2 * heads].bitcast(F32)
                    u = wp.tile([P, KC, heads], F32, tag=f"u{layer}")
                    nc.vector.tensor_tensor(
                        out=u[:, 0:kn, :], in0=als_v,
                        in1=_bap(aldt[:, l, :], [(0, kn), (1, heads)]),
                        op=OP.add)
                    nc.vector.scalar_tensor_tensor(
                        out=u[:, 0:kn, :], in0=u[:, 0:kn, :], scalar=0.2,
                        in1=u[:, 0:kn, :], op0=OP.mult, op1=OP.max)
                    exf = wp.tile([P, KC, heads], F32, tag=f"ex{layer}")
                    nc.scalar.activation(exf[:, 0:kn, :], u[:, 0:kn, :], AF.Exp)
                    dt_ = wp.tile([P, heads], F32, tag=f"dt{layer}")
                    red = den if k0 == 0 else dt_
                    if heads > 1:
                        nc.vector.tensor_reduce(
                            out=red[:], in_=_bap(exf[:], [(1, heads), (heads, kn)]),
                            axis=mybir.AxisListType.X, op=OP.add)
                    else:
                        nc.vector.tensor_reduce(
                            out=red[:], in_=_bap(exf[:], [(1, kn)]),
                            axis=mybir.AxisListType.X, op=OP.add)
                    if k0:
                        nc.vector.tensor_add(den[:], den[:], dt_[:])
                    w = wq.tile([P, KC, C], BF, tag=f"w{layer}")
                    nc.scalar.activation(
                        bass.AP(w[:].tensor, w[:].offset,
                                [w[:].ap[0], [C, kn], [HID, heads], [1, HID]]),
                        _bap(u[:], [(heads, kn), (1, heads), (0, HID)]),
                        AF.Exp)
                    nc.vector.tensor_tensor(
                        out=w[:, 0:kn, :],
                        in0=(_bap(gt[:], [(tcols, kn), (HID, heads), (1, HID)])
                             if heads > 1 else
                             _bap(gt[:], [(tcols, kn), (1, C)])),
                        in1=w[:, 0:kn, :],
                        op=OP.mult)
                    for k in range(kn):
                        nc.tensor.matmul(psA[:], lhsT=ident[:], rhs=w[:, k, :],
                                         start=(k0 + k == 0),
                                         stop=(k0 + k == ghat[l] - 1))
                    if last:
                        # normalize + extract to SBUF bf16, release PSUM
                        nc.vector.tensor_scalar_add(out=den[:], in0=den[:],
                                                    scalar1=1e-30)
                        denr = wp.tile([P, heads], F32, tag=f"dr{layer}")
                        nc.vector.reciprocal(denr[:], den[:])
                        hb = hp.tile([P, C], BF, tag=f"h{layer}_{l % GRP}")
                        for h in range(heads):
                            nc.vector.tensor_scalar_mul(
                                out=hb[:, h * HID:(h + 1) * HID],
                                in0=psA[:, h * HID:(h + 1) * HID],
                                scalar1=denr[:, h:h + 1])
                        state[f"hb{l}"] = hb
                    if epi is not None:
                        epi(grp, state)

                return prep, compute

            # ---------- batched epilogue: layer 1 ----------
            def epi1(grp, state):
                gi = grp[0] // GRP
                hbs = [state.pop(f"hb{l}") for l in grp]
                xcs, rstds = [], []
                for hb in hbs:
                    nc.vector.tensor_tensor(out=hb[:], in0=hb[:],
                                            in1=ccb[:, CC_BIAS1:CC_BIAS1 + 512],
                                            op=OP.add)
                    mu = wp.tile([P, 1], F32, tag="e1mu")
                    nc.vector.tensor_reduce(out=mu[:], in_=hb[:],
                                            axis=mybir.AxisListType.X, op=OP.add)
                    nc.scalar.mul(mu[:], mu[:], 1.0 / 512)
                    nc.vector.tensor_scalar_sub(out=hb[:], in0=hb[:], scalar1=mu[:])
                    xcs.append(hb)
                # ACT table: Square
                for bi, xc in enumerate(xcs):
                    sq = wp.tile([P, 512], BF, tag="e1sq", name=f"e1sq{bi}")
                    ss = wp.tile([P, 1], F32, tag=f"e1ss{bi}", name=f"e1ss{bi}")
                    nc.scalar.activation(sq[:], xc[:], AF.Square, accum_out=ss[:])
                    rstd = wp.tile([P, 1], F32, tag=f"e1rs{bi}", name=f"e1rs{bi}")
                    rstds.append((ss, rstd))
                # ACT table: Rsqrt
                for ss, rstd in rstds:
                    nc.scalar.activation(rstd[:], ss[:], AF.Sqrt,
                                         bias=eps_t[:], scale=1.0 / 512)
                for ss, rstd in rstds:
                    nc.vector.reciprocal(rstd[:], rstd[:])
                for xc, (ss, rstd) in zip(xcs, rstds):
                    nc.vector.tensor_scalar_mul(out=xc[:], in0=xc[:],
                                                scalar1=rstd[:])
                    nc.vector.tensor_mul(out=xc[:], in0=xc[:],
                                         in1=ccb[:, CC_G1:CC_G1 + 512])
                    nc.vector.tensor_tensor(out=xc[:], in0=xc[:],
                                            in1=ccb[:, CC_B1:CC_B1 + 512],
                                            op=OP.add)
                # ACT table: Gelu
                h1bs = []
                for bi, xc in enumerate(xcs):
                    h1b = wp.tile([P, 512], BF, tag=f"h1b{bi}", name=f"h1b{bi}")
                    if sim_safe:
                        _gelu_tanh(xc, h1b, 512)
                    else:
                        nc.scalar.activation(h1b[:], xc[:], AF.Gelu)
                    h1bs.append(h1b)
                # W2 matmul + t2 build + AG2 chunk
                for l, h1b in zip(grp, h1bs):
                    ps3 = pss.tile([P, 130], F32, tag="mm2")
                    for cch in range(4):
                        hT = wp.tile([P, P], BF, tag="hT")
                        transpose_to(hT[:], h1b[:, cch * P:(cch + 1) * P])
                        nc.tensor.matmul(ps3[:], lhsT=hT[:], rhs=w2s[:, cch, :],
                                         start=(cch == 0), stop=(cch == 3))
                    t2 = wp.tile([P, T2COLS], BF, tag="t2")
                    nc.vector.memset(t2[:, 130:T2COLS], 0)
                    nc.vector.tensor_copy(out=t2[:, 0:128], in_=ps3[:, 0:128])
                    nc.vector.tensor_tensor(
                        out=t2[:, 128:130].bitcast(F32), in0=ps3[:, 128:129],
                        in1=afix[:, l, 4:5], op=OP.add)
                    nc.vector.tensor_copy(out=ald2[:, l, :], in_=ps3[:, 129:130])
                    nc.sync.dma_start(ag2_in[l * P:(l + 1) * P, :], t2[:])
                r0, r1 = gi * NPB // NAG, (gi + 1) * NPB // NAG
                nc.gpsimd.collective_compute(
                    "AllGather", OP.bypass, replica_groups=rg,
                    ins=[ag2_in[r0:r1, :].opt()],
                    outs=[ag2_out[gi * NPAD // NAG:
                                  (gi + 1) * NPAD // NAG, :].opt()])

            def _gelu_tanh(xin, out_bf, D):
                x3 = wp.tile([P, D], F32, tag="gx3")
                nc.scalar.activation(x3[:], xin[:], AF.Square)
                nc.vector.tensor_mul(out=x3[:], in0=x3[:], in1=xin[:])
                nc.vector.scalar_tensor_tensor(out=x3[:], in0=x3[:],
                                               scalar=0.044715, in1=xin[:],
                                               op0=OP.mult, op1=OP.add)
                th = wp.tile([P, D], F32, tag="gth")
                nc.scalar.activation(th[:], x3[:], AF.Tanh,
                                     scale=0.797884560802865)
                nc.vector.tensor_scalar_add(out=th[:], in0=th[:], scalar1=1.0)
                nc.vector.tensor_mul(out=th[:], in0=th[:], in1=xin[:])
                nc.scalar.mul(out_bf[:], th[:], 0.5)

            # ---------- batched epilogue: layer 2 + output head ----------
            def epi2(grp, state):
                hbs = [state.pop(f"hb{l}") for l in grp]
                xcs, rstds = [], []
                for hb in hbs:
                    nc.vector.tensor_tensor(out=hb[:], in0=hb[:],
                                            in1=ccb[:, CC_BIAS2:CC_BIAS2 + 128],
                                            op=OP.add)
                    mu = wp.tile([P, 1], F32, tag="e2mu")
                    nc.vector.tensor_reduce(out=mu[:], in_=hb[:],
                                            axis=mybir.AxisListType.X, op=OP.add)
                    nc.scalar.mul(mu[:], mu[:], 1.0 / 128)
                    nc.vector.tensor_scalar_sub(out=hb[:], in0=hb[:], scalar1=mu[:])
                    xcs.append(hb)
                for bi, xc in enumerate(xcs):
                    sq = wp.tile([P, 128], BF, tag="e2sq", name=f"e2sq{bi}")
                    ss = wp.tile([P, 1], F32, tag=f"e2ss{bi}", name=f"e2ss{bi}")
                    nc.scalar.activation(sq[:], xc[:], AF.Square, accum_out=ss[:])
                    rstd = wp.tile([P, 1], F32, tag=f"e2rs{bi}", name=f"e2rs{bi}")
                    rstds.append((ss, rstd))
                for ss, rstd in rstds:
                    nc.scalar.activation(rstd[:], ss[:], AF.Sqrt,
                                         bias=eps_t[:], scale=1.0 / 128)
                for ss, rstd in rstds:
                    nc.vector.reciprocal(rstd[:], rstd[:])
                for xc, (ss, rstd) in zip(xcs, rstds):
                    nc.vector.tensor_scalar_mul(out=xc[:], in0=xc[:],
                                                scalar1=rstd[:])
                    nc.vector.tensor_mul(out=xc[:], in0=xc[:],
                                         in1=ccb[:, CC_G2:CC_G2 + 128])
                    nc.vector.tensor_tensor(out=xc[:], in0=xc[:],
                                            in1=ccb[:, CC_B2:CC_B2 + 128],
                                            op=OP.add)
                h2bs = []
                for bi, xc in enumerate(xcs):
                    h2b = wp.tile([P, 128], BF, tag=f"h2b{bi}", name=f"h2b{bi}")
                    if sim_safe:
                        _gelu_tanh(xc, h2b, 128)
                    else:
                        nc.scalar.activation(h2b[:], xc[:], AF.Gelu)
                    h2bs.append(h2b)
                zs = []
                for l, h2b in zip(grp, h2bs):
                    hoT = wp.tile([P, P], BF, tag="hoT")
                    transpose_to(hoT[:], h2b[:])
                    pso_t = pss.tile([P, 130], F32, tag="mm2")
                    pso = pso_t[:, 0:D_OUT]
                    nc.tensor.matmul(pso[:], lhsT=hoT[:], rhs=wos[:],
                                     start=True, stop=True)
                    z = hp.tile([P, D_OUT], F32, tag=f"z_{len(zs)}")
                    nc.vector.tensor_tensor(out=z[:], in0=pso[:],
                                            in1=ccb[:, CC_BO:CC_BO + 32],
                                            op=OP.add)
                    m = wp.tile([P, 1], F32, tag="zm")
                    nc.vector.tensor_reduce(out=m[:], in_=z[:],
                                            axis=mybir.AxisListType.X, op=OP.max)
                    nc.vector.tensor_scalar_sub(out=z[:], in0=z[:], scalar1=m[:])
                    zs.append((l, z))
                sds = []
                for bi, (l, z) in enumerate(zs):
                    ez = wp.tile([P, D_OUT], F32, tag="ez", name=f"ez{bi}")
                    sden = wp.tile([P, 1], F32, tag=f"sden{bi}", name=f"sden{bi}")
                    nc.scalar.activation(ez[:], z[:], AF.Exp, accum_out=sden[:])
                    sds.append(sden)
                lnds = []
                for bi, sden in enumerate(sds):
                    lnd = wp.tile([P, 1], F32, tag=f"lnd{bi}", name=f"lnd{bi}")
                    nc.scalar.activation(lnd[:], sden[:], AF.Ln)
                    lnds.append(lnd)
                for (l, z), lnd in zip(zs, lnds):
                    res = wp.tile([P, D_OUT], F32, tag="res")
                    nc.vector.tensor_scalar_sub(out=res[:], in0=z[:],
                                                scalar1=lnd[:])
                    nc.sync.dma_start(out[l * P:(l + 1) * P, :], res[:])

            # ---------- emit ----------
            # NOTE: preps capture their gather-source deps at emission time,
            # so every AllGather writing a table must be emitted before the
            # first prep that reads it (layer pipelines run separately).
            phase0()
            groups2 = [[g * GRP + i for i in range(GRP)] for g in grp_order2]
            gat_layer(1, groups2, T1COLS, H1, epi1)
            run_jobs()
            groups4 = [[g * GRP + i for i in range(GRP)] for g in range(NAG)]
            gat_layer(2, groups4, T2COLS, H2, epi2)
            run_jobs()

    nc.compile()
    return nc


_CACHE = {}
_LAST_RUN = {}


def kernel(x, edge_index, g_in, b_in, W1, att1_s, att1_d, bias1, g1, b1,
           W2, att2_s, att2_d, bias2, g2, b2, Wo, bo):
    prep = prepare_inputs(x, edge_index)
    wts = prepare_weights(W1, att1_s, att1_d, bias1, g1, b1, g_in, b_in,
                          W2, att2_s, att2_d, bias2, g2, b2, Wo, bo)

    key = tuple(prep["ghat"])
    if key not in _CACHE:
        _CACHE[key] = build_program(prep["ghat"])
    nc = _CACHE[key]

    in_maps = []
    for c in range(NCORES):
        in_maps.append({
            "x_own": prep["x_own"][c],
            "idxw": prep["idxw"][c],
            "alsfix": prep["alsfix"][c],
            "w1ext": wts["w1ext"],
            "w2ext": wts["w2ext"].astype(BF16),
            "woext": wts["woext"],
            "colconst": wts["colconst"].astype(BF16),
        })

    _LAST_RUN.update(nc=nc, in_maps=in_maps, prep=prep)
    res = bass_utils.run_bass_kernel_spmd(nc, in_maps,
                                          core_ids=list(range(NCORES)))
    outs = [res.results[c]["out"] for c in range(NCORES)]

    newid = prep["newid"]
    blk = newid // P
    core = blk % NCORES
    row = (blk // NCORES) * P + newid % P
    full = np.empty((N, D_OUT), dtype=np.float32)
    for c in range(NCORES):
        sel = core == c
        full[sel] = outs[c][row[sel]]
    return full



# revision 6
# speedup vs baseline: 1.1324x; 1.1324x over previous
"""Distributed 2-layer GAT kernel for 8 Trainium2 NeuronCores — v2.

Same host-side graph preprocessing as v1 (degree-sorted relabeling, dst-major
edge slots, SPMD per-core programs keyed by the compile-time degree schedule
ghat).  Device-side restructured around the measured bottlenecks:

  * dma_gather descriptor prep on the Q7 (~8ns/row) is the critical resource:
    gathers are issued as prepare_only preps with a PA-deep software pipeline
    so prep for chunk i+PA overlaps trigger+DMA+compute for chunk i, and the
    first preps run under the phase-0/AllGather window.
  * AllGather outputs use addr_space="Shared" (single shared table instead of
    8 replicated copies) when USE_SHARED_AG is set.
  * Per-block epilogues are batched in groups of 5 blocks (one AG2 chunk) and
    ordered op-type-major so the scalar engine's activation table is loaded
    once per function per group instead of once per block.
  * LN uses Rsqrt (one ACT op) and bf16 intermediates.
  * Phase-2 groups run heaviest-first so the last AG2 chunk covers the
    lightest blocks (short tail before phase 4).
"""
import sys

sys.path.insert(0, "/opt/trn_rl_repo")

import numpy as np
import ml_dtypes

from concourse import bass, bacc, tile, mybir
from concourse import bass_utils
from concourse.masks import make_identity

BF16 = ml_dtypes.bfloat16
F32 = mybir.dt.float32
BF = mybir.dt.bfloat16
I16 = mybir.dt.int16
AF = mybir.ActivationFunctionType
OP = mybir.AluOpType

# problem constants
N, E = 20000, 320000
D_IN, HID, D_OUT = 128, 128, 32
H1, H2 = 4, 1
EPS = 1e-5

NCORES = 8
P = 128
NPAD = 20480
NBLK_G = NPAD // P
NPB = NPAD // NCORES    # 2560
NBLK = NPB // P         # 20 blocks per core
NEG = -1e9

T1COLS = 640            # L1 table row (bf16): 512 feats | 4 f32 a_src | pad
T2COLS = 256            # L2 table row (bf16): 128 feats | 1 f32 a_src | pad
KC = 16                 # max in-edge slots per gather call
NAG = 4                 # AllGather chunks
GRP = NBLK // NAG       # blocks per epilogue/AG group (5)
NSWQ = 4                # SWDGE queues
PA = 3                  # gather prep-ahead depth (gp bufs = PA + 1)

USE_SHARED_AG = True
SINGLE_PACKET = False     # True crashes NRT with 1280B rows (packet limit)
SP_BY_LAYER = {1: False, 2: False}  # single_packet=True crashes NRT (both 1280B and 512B rows)
USE_PREP = True

CC_GIN, CC_BIN = 0, 128
CC_G1, CC_B1, CC_BIAS1 = 256, 768, 1280
CC_G2, CC_B2, CC_BIAS2 = 1792, 1920, 2048
CC_BO = 2176
NCC = 2208


def _tid(n):
    blk = n // P
    c = blk % NCORES
    l = blk // NCORES
    cb = NBLK // NAG
    return ((l // cb) * (NPAD // NAG) + c * (NPB // NAG)
            + (l % cb) * P + n % P)


def prepare_inputs(x, edge_index):
    x = np.asarray(x, dtype=np.float32)
    ei = np.asarray(edge_index)
    src = np.concatenate([ei[0], np.arange(N, dtype=ei.dtype)]).astype(np.int64)
    dst = np.concatenate([ei[1], np.arange(N, dtype=ei.dtype)]).astype(np.int64)

    deg = np.bincount(dst, minlength=N)
    order = np.argsort(deg, kind="stable")
    newid = np.empty(N, dtype=np.int64)
    newid[order] = np.arange(N) + (NPAD - N)

    degp = np.zeros(NPAD, dtype=np.int64)
    degp[newid] = deg
    gmax = degp.reshape(NBLK_G, P).max(axis=1)
    ghat = gmax.reshape(NBLK, NCORES).max(axis=1)
    S = int(P * ghat.sum())

    nd = newid[dst]
    csr_order = np.argsort(nd, kind="stable")
    nsrc_sorted = newid[src[csr_order]]
    indptr = np.zeros(NPAD + 1, dtype=np.int64)
    np.cumsum(np.bincount(nd, minlength=NPAD), out=indptr[1:])

    tid_of = _tid(np.arange(NPAD))

    goff = np.zeros(NBLK, dtype=np.int64)
    goff[1:] = np.cumsum(ghat)[:-1]

    idxw = np.zeros((NCORES, P, S // 16), dtype=np.int16)
    x_own = np.zeros((NCORES, NPB, D_IN), dtype=np.float32)
    alsfix = np.zeros((NCORES, NPB, 8), dtype=np.float32)

    inv_new = np.full(NPAD, -1, dtype=np.int64)
    inv_new[newid] = np.arange(N)

    for c in range(NCORES):
        gblk = np.arange(NBLK) * NCORES + c
        nid = (gblk[:, None] * P + np.arange(P)).reshape(-1)
        ov = inv_new[nid]
        real = ov >= 0
        x_own[c][real] = x[ov[real]]
        alsfix[c][~real, :] = NEG

        idx_flat = np.zeros(S, dtype=np.int16)
        for l in range(NBLK):
            d0 = nid[l * P:(l + 1) * P]
            base = goff[l] * P
            for p in range(P):
                d = d0[p]
                s0, s1 = indptr[d], indptr[d + 1]
                ks = np.arange(s1 - s0)
                idx_flat[base + ks * P + p] = tid_of[nsrc_sorted[s0:s1]]
        idxw[c] = np.tile(idx_flat.reshape(S // 16, 16).T, (NCORES, 1))

    return {
        "ghat": [int(g) for g in ghat],
        "S": S,
        "idxw": idxw,
        "x_own": x_own,
        "alsfix": alsfix,
        "newid": newid,
    }


def prepare_weights(W1, att1_s, att1_d, bias1, g1, b1, g_in, b_in,
                    W2, att2_s, att2_d, bias2, g2, b2, Wo, bo):
    W1 = np.asarray(W1, np.float32)
    W2 = np.asarray(W2, np.float32)
    w1ext = np.zeros((D_IN, 520), dtype=BF16)
    w1ext[:, :512] = W1
    W1h = W1.reshape(D_IN, H1, HID)
    w1ext[:, 512:516] = np.einsum("khc,hc->kh", W1h, np.asarray(att1_s, np.float32))
    w1ext[:, 516:520] = np.einsum("khc,hc->kh", W1h, np.asarray(att1_d, np.float32))

    w2e = np.zeros((4 * HID, 130), dtype=np.float32)
    w2e[:, :128] = W2
    w2e[:, 128] = W2 @ np.asarray(att2_s, np.float32)[0]
    w2e[:, 129] = W2 @ np.asarray(att2_d, np.float32)[0]
    w2ext = np.ascontiguousarray(
        w2e.reshape(4, P, 130).transpose(1, 0, 2)).astype(BF16)

    woext = np.asarray(Wo, np.float32).astype(BF16)

    cc = np.zeros(NCC, dtype=np.float32)
    cc[CC_GIN:CC_GIN + 128] = g_in
    cc[CC_BIN:CC_BIN + 128] = b_in
    cc[CC_G1:CC_G1 + 512] = g1
    cc[CC_B1:CC_B1 + 512] = b1
    cc[CC_BIAS1:CC_BIAS1 + 512] = bias1
    cc[CC_G2:CC_G2 + 128] = g2
    cc[CC_B2:CC_B2 + 128] = b2
    cc[CC_BIAS2:CC_BIAS2 + 128] = bias2
    cc[CC_BO:CC_BO + 32] = bo
    colconst = np.tile(cc[None, :], (P, 1))

    return {"w1ext": w1ext, "w2ext": w2ext.reshape(P, 4 * 130),
            "woext": woext, "colconst": colconst}


def _bap(ap, dims):
    return bass.AP(ap.tensor, ap.offset, [ap.ap[0]] + [list(d) for d in dims])


def build_program(ghat, num_devices=NCORES, sim_safe=False):
    S = int(P * sum(ghat))
    goff = np.zeros(NBLK, dtype=np.int64)
    goff[1:] = np.cumsum(ghat)[:-1]

    nc = bacc.Bacc("TRN2", target_bir_lowering=False, debug=False,
                   num_devices=num_devices, num_swdge_queues=NSWQ)

    x_own = nc.dram_tensor("x_own", [NPB, D_IN], F32, kind="ExternalInput")
    idxw = nc.dram_tensor("idxw", [P, S // 16], I16, kind="ExternalInput")
    alsfix = nc.dram_tensor("alsfix", [NPB, 8], F32, kind="ExternalInput")
    w1ext = nc.dram_tensor("w1ext", [D_IN, 520], BF, kind="ExternalInput")
    w2ext = nc.dram_tensor("w2ext", [P, 4 * 130], BF, kind="ExternalInput")
    woext = nc.dram_tensor("woext", [P, D_OUT], BF, kind="ExternalInput")
    colconst = nc.dram_tensor("colconst", [P, NCC], BF, kind="ExternalInput")
    out = nc.dram_tensor("out", [NPB, D_OUT], F32, kind="ExternalOutput")

    if USE_SHARED_AG:
        ag1_out_t = nc.dram_tensor("ag1o", [NPAD, T1COLS], BF,
                                   kind="Internal", addr_space="Shared")
        ag2_out_t = nc.dram_tensor("ag2o", [NPAD, T2COLS], BF,
                                   kind="Internal", addr_space="Shared")

    rg = [list(range(num_devices))]

    # processing order for phase 2: heaviest group first
    grp_w = [sum(ghat[g * GRP:(g + 1) * GRP]) for g in range(NAG)]
    grp_order2 = sorted(range(NAG), key=lambda g: -grp_w[g])

    with tile.TileContext(nc) as tc:
        with (
            tc.tile_pool(name="cst", bufs=1) as cst,
            tc.tile_pool(name="wp", bufs=2) as wp,
            tc.tile_pool(name="hp", bufs=1) as hp,
            tc.tile_pool(name="wq", bufs=2) as wq,
            tc.tile_pool(name="gp", bufs=PA + 1) as gp,
            tc.tile_pool(name="ps", bufs=2, space="PSUM") as ps,
            tc.tile_pool(name="pss", bufs=2, space="PSUM") as pss,
            tc.tile_pool(name="dram", bufs=1, space="DRAM") as dram,
        ):
            # ---- constants (idx first: unblocks gather preps) ----
            idx_sb = cst.tile([P, S // 16], I16)
            nc.sync.dma_start(idx_sb[:], idxw[:])
            ident = cst.tile([P, P], BF)
            make_identity(nc, ident[:])
            w1s = cst.tile([P, 520], BF)
            nc.sync.dma_start(w1s[:], w1ext[:])
            w2s = cst.tile([P, 4, 130], BF)
            nc.sync.dma_start(w2s[:], w2ext[:])
            wos = cst.tile([P, D_OUT], BF)
            nc.sync.dma_start(wos[:], woext[:])
            ccb = cst.tile([P, NCC], BF)
            nc.sync.dma_start(ccb[:], colconst[:])
            afix = cst.tile([P, NBLK, 8], F32)
            nc.sync.dma_start(
                afix[:], bass.AP(alsfix.ap().tensor, 0,
                                 [[8, P], [8 * P, NBLK], [1, 8]]))
            eps_t = cst.tile([P, 1], F32)
            nc.vector.memset(eps_t[:], EPS)

            ald1 = cst.tile([P, NBLK, H1], F32)
            ald2 = cst.tile([P, NBLK, 1], F32)

            ag1_in = dram.tile([NPB, T1COLS], BF)
            ag2_in = dram.tile([NPB, T2COLS], BF)
            if USE_SHARED_AG:
                ag1_out = ag1_out_t
                ag2_out = ag2_out_t
            else:
                ag1_out = dram.tile([NPAD, T1COLS], BF)
                ag2_out = dram.tile([NPAD, T2COLS], BF)

            dsem = [nc.alloc_semaphore(f"gdma{q}") for q in range(NSWQ)]

            def transpose_to(dst_bf, src_bf):
                pst = pss.tile([P, P], BF, tag="tp")
                nc.tensor.transpose(out=pst[:], in_=src_bf, identity=ident[:])
                nc.vector.tensor_copy(out=dst_bf, in_=pst[:])

            # ---------- phase 0: LN0 + W1 matmul, build L1 table shard ----
            def phase0():
                for t in range(NBLK):
                    xt = wp.tile([P, D_IN], F32, tag="xt")
                    nc.sync.dma_start(xt[:], x_own[t * P:(t + 1) * P, :])
                    # LN in f32 input precision, bf16 out
                    mu = wp.tile([P, 1], F32, tag="p0mu")
                    nc.vector.tensor_reduce(out=mu[:], in_=xt[:],
                                            axis=mybir.AxisListType.X, op=OP.add)
                    nc.scalar.mul(mu[:], mu[:], 1.0 / D_IN)
                    xc = wp.tile([P, D_IN], F32, tag="p0xc")
                    nc.vector.tensor_scalar_sub(out=xc[:], in0=xt[:], scalar1=mu[:])
                    sq = wp.tile([P, D_IN], F32, tag="p0sq")
                    ss = wp.tile([P, 1], F32, tag="p0ss")
                    nc.scalar.activation(sq[:], xc[:], AF.Square, accum_out=ss[:])
                    sd = wp.tile([P, 1], F32, tag="p0sd")
                    nc.scalar.activation(sd[:], ss[:], AF.Sqrt,
                                         bias=eps_t[:], scale=1.0 / D_IN)
                    rstd = wp.tile([P, 1], F32, tag="p0rs")
                    nc.vector.reciprocal(rstd[:], sd[:])
                    nc.vector.tensor_scalar_mul(out=xc[:], in0=xc[:], scalar1=rstd[:])
                    nc.vector.tensor_mul(out=xc[:], in0=xc[:],
                                         in1=ccb[:, CC_GIN:CC_GIN + 128])
                    xnb = wp.tile([P, D_IN], BF, tag="xnb")
                    nc.vector.tensor_tensor(out=xnb[:], in0=xc[:],
                                            in1=ccb[:, CC_BIN:CC_BIN + 128],
                                            op=OP.add)
                    xT = wp.tile([P, P], BF, tag="xT")
                    transpose_to(xT[:], xnb[:])
                    ps1 = ps.tile([P, 512], F32, tag="big")
                    nc.tensor.matmul(ps1[:], lhsT=xT[:], rhs=w1s[:, 0:512],
                                     start=True, stop=True)
                    ps2_t = pss.tile([P, 130], F32, tag="mm2")
                    ps2 = ps2_t[:, 0:8]
                    nc.tensor.matmul(ps2[:], lhsT=xT[:], rhs=w1s[:, 512:520],
                                     start=True, stop=True)
                    tt = wp.tile([P, T1COLS], BF, tag="tt")
                    nc.vector.memset(tt[:, 520:T1COLS], 0)
                    nc.vector.tensor_copy(out=tt[:, 0:512], in_=ps1[:])
                    nc.vector.tensor_tensor(
                        out=tt[:, 512:520].bitcast(F32), in0=ps2[:, 0:4],
                        in1=afix[:, t, 0:4], op=OP.add)
                    nc.vector.tensor_copy(out=ald1[:, t, :], in_=ps2[:, 4:8])
                    nc.sync.dma_start(ag1_in[t * P:(t + 1) * P, :], tt[:])
                    if t % GRP == GRP - 1:
                        j = t // GRP
                        r0, r1 = j * NPB // NAG, (j + 1) * NPB // NAG
                        nc.gpsimd.collective_compute(
                            "AllGather", OP.bypass, replica_groups=rg,
                            ins=[ag1_in[r0:r1, :].opt()],
                            outs=[ag1_out[j * NPAD // NAG:
                                          (j + 1) * NPAD // NAG, :].opt()])

            # ---------- gather pipeline machinery ----------
            jobs = []   # list of (prep_fn, compute_fn)
            qrr = [0]

            def run_jobs(head=0):
                n = len(jobs)
                if not USE_PREP:
                    for i in range(n):
                        if i >= head:
                            jobs[i][0]()
                        jobs[i][1]()
                else:
                    for i in range(n + PA):
                        if head <= i < n:
                            jobs[i][0]()
                        j = i - PA
                        if 0 <= j < n:
                            jobs[j][1]()
                jobs.clear()

            # ---------- phase 2: GAT layer 1 ----------
            def gat_layer(layer, blocks_by_group, tcols, heads, epilogue):
                """Queue gather+aggregate jobs for `blocks_by_group` and
                attach the batched epilogue after each group."""
                tag = f"g{layer}"
                table = (ag1_out[0:NPAD, :] if layer == 1
                         else ag2_out[0:NPAD, :])
                aldt = ald1 if layer == 1 else ald2
                state = {}

                for gi, grp in enumerate(blocks_by_group):
                    for bi, l in enumerate(grp):
                        g = ghat[l]
                        chunks = []
                        k0 = 0
                        while k0 < g:
                            kn = min(KC, g - k0)
                            chunks.append((k0, kn))
                            k0 += kn
                        for ci, (k0, kn) in enumerate(chunks):
                            last_of_grp = (bi == len(grp) - 1
                                           and ci == len(chunks) - 1)
                            jobs.append(_make_job(
                                layer, tag, table, aldt, l, k0, kn,
                                ci == 0, k0 + kn >= g, tcols, heads, state,
                                epilogue if last_of_grp else None, grp))

            def _make_job(layer, tag, table, aldt, l, k0, kn, first, last,
                          tcols, heads, state, epi, grp):
                gt = gp.tile([P, KC, tcols], BF, tag=tag)
                q = qrr[0] % NSWQ
                qrr[0] += 1

                def prep():
                    if USE_PREP:
                        nc.gpsimd.dma_gather(
                            gt[:, 0:kn, :], table,
                            idx_sb[:, 8 * (int(goff[l]) + k0):
                                   8 * (int(goff[l]) + k0 + kn)],
                            P * kn, P * kn, tcols, prepare_only=True,
                            sem=dsem[q], single_packet=SP_BY_LAYER[layer],
                            queue_num=q)
                    else:
                        nc.gpsimd.dma_gather(
                            gt[:, 0:kn, :], table,
                            idx_sb[:, 8 * (int(goff[l]) + k0):
                                   8 * (int(goff[l]) + k0 + kn)],
                            P * kn, P * kn, tcols,
                            single_packet=SP_BY_LAYER[layer], queue_num=q)

                def compute():
                    if USE_PREP:
                        nc.gpsimd.trigger_dma(count=None, queue_num=q)
                    C = 512 if layer == 1 else 128
                    fb = 512 if layer == 1 else 128   # feature bytes offset
                    if first:
                        psA_t = ps.tile([P, 512], F32, tag="big", name=f"psA{layer}_{l}")
                        den_t = wp.tile([P, heads], F32, tag=f"den{layer}",
                                        name=f"den{layer}_{l}")
                        state["psA"] = psA_t
                        state["den"] = den_t
                    psA = state["psA"][:, 0:C]
                    den = state["den"]
                    als_v = gt[:, 0:kn, fb:fb + 2 * heads].bitcast(F32)
                    u = wp.tile([P, KC, heads], F32, tag=f"u{layer}")
                    nc.vector.tensor_tensor(
                        out=u[:, 0:kn, :], in0=als_v,
                        in1=_bap(aldt[:, l, :], [(0, kn), (1, heads)]),
                        op=OP.add)
                    nc.vector.scalar_tensor_tensor(
                        out=u[:, 0:kn, :], in0=u[:, 0:kn, :], scalar=0.2,
                        in1=u[:, 0:kn, :], op0=OP.mult, op1=OP.max)
                    exf = wp.tile([P, KC, heads], F32, tag=f"ex{layer}")
                    nc.scalar.activation(exf[:, 0:kn, :], u[:, 0:kn, :], AF.Exp)
                    dt_ = wp.tile([P, heads], F32, tag=f"dt{layer}")
                    red = den if k0 == 0 else dt_
                    if heads > 1:
                        nc.vector.tensor_reduce(
                            out=red[:], in_=_bap(exf[:], [(1, heads), (heads, kn)]),
                            axis=mybir.AxisListType.X, op=OP.add)
                    else:
                        nc.vector.tensor_reduce(
                            out=red[:], in_=_bap(exf[:], [(1, kn)]),
                            axis=mybir.AxisListType.X, op=OP.add)
                    if k0:
                        nc.vector.tensor_add(den[:], den[:], dt_[:])
                    w = wq.tile([P, KC, C], BF, tag=f"w{layer}")
                    nc.scalar.activation(
                        bass.AP(w[:].tensor, w[:].offset,
                                [w[:].ap[0], [C, kn], [HID, heads], [1, HID]]),
                        _bap(u[:], [(heads, kn), (1, heads), (0, HID)]),
                        AF.Exp)
                    nc.vector.tensor_tensor(
                        out=w[:, 0:kn, :],
                        in0=(_bap(gt[:], [(tcols, kn), (HID, heads), (1, HID)])
                             if heads > 1 else
                             _bap(gt[:], [(tcols, kn), (1, C)])),
                        in1=w[:, 0:kn, :],
                        op=OP.mult)
                    for k in range(kn):
                        nc.tensor.matmul(psA[:], lhsT=ident[:], rhs=w[:, k, :],
                                         start=(k0 + k == 0),
                                         stop=(k0 + k == ghat[l] - 1))
                    if last:
                        # normalize + extract to SBUF bf16, release PSUM
                        nc.vector.tensor_scalar_add(out=den[:], in0=den[:],
                                                    scalar1=1e-30)
                        denr = wp.tile([P, heads], F32, tag=f"dr{layer}")
                        nc.vector.reciprocal(denr[:], den[:])
                        hb = hp.tile([P, C], BF, tag=f"h{layer}_{l % GRP}")
                        for h in range(heads):
                            nc.vector.tensor_scalar_mul(
                                out=hb[:, h * HID:(h + 1) * HID],
                                in0=psA[:, h * HID:(h + 1) * HID],
                                scalar1=denr[:, h:h + 1])
                        state[f"hb{l}"] = hb
                    if epi is not None:
                        epi(grp, state)

                return prep, compute

            # ---------- batched epilogue: layer 1 ----------
            def epi1(grp, state):
                gi = grp[0] // GRP
                hbs = [state.pop(f"hb{l}") for l in grp]
                xcs, rstds = [], []
                for hb in hbs:
                    nc.vector.tensor_tensor(out=hb[:], in0=hb[:],
                                            in1=ccb[:, CC_BIAS1:CC_BIAS1 + 512],
                                            op=OP.add)
                    mu = wp.tile([P, 1], F32, tag="e1mu")
                    nc.vector.tensor_reduce(out=mu[:], in_=hb[:],
                                            axis=mybir.AxisListType.X, op=OP.add)
                    nc.scalar.mul(mu[:], mu[:], 1.0 / 512)
                    nc.vector.tensor_scalar_sub(out=hb[:], in0=hb[:], scalar1=mu[:])
                    xcs.append(hb)
                # ACT table: Square
                for bi, xc in enumerate(xcs):
                    sq = wp.tile([P, 512], BF, tag="e1sq", name=f"e1sq{bi}")
                    ss = wp.tile([P, 1], F32, tag=f"e1ss{bi}", name=f"e1ss{bi}")
                    nc.scalar.activation(sq[:], xc[:], AF.Square, accum_out=ss[:])
                    rstd = wp.tile([P, 1], F32, tag=f"e1rs{bi}", name=f"e1rs{bi}")
                    rstds.append((ss, rstd))
                # ACT table: Rsqrt
                for ss, rstd in rstds:
                    nc.scalar.activation(rstd[:], ss[:], AF.Sqrt,
                                         bias=eps_t[:], scale=1.0 / 512)
                for ss, rstd in rstds:
                    nc.vector.reciprocal(rstd[:], rstd[:])
                for xc, (ss, rstd) in zip(xcs, rstds):
                    nc.vector.tensor_scalar_mul(out=xc[:], in0=xc[:],
                                                scalar1=rstd[:])
                    nc.vector.tensor_mul(out=xc[:], in0=xc[:],
                                         in1=ccb[:, CC_G1:CC_G1 + 512])
                    nc.vector.tensor_tensor(out=xc[:], in0=xc[:],
                                            in1=ccb[:, CC_B1:CC_B1 + 512],
                                            op=OP.add)
                # ACT table: Gelu
                h1bs = []
                for bi, xc in enumerate(xcs):
                    h1b = wp.tile([P, 512], BF, tag=f"h1b{bi}", name=f"h1b{bi}")
                    if sim_safe:
                        _gelu_tanh(xc, h1b, 512)
                    else:
                        nc.scalar.activation(h1b[:], xc[:], AF.Gelu)
                    h1bs.append(h1b)
                # W2 matmul + t2 build + AG2 chunk
                for l, h1b in zip(grp, h1bs):
                    ps3 = pss.tile([P, 130], F32, tag="mm2")
                    for cch in range(4):
                        hT = wp.tile([P, P], BF, tag="hT")
                        transpose_to(hT[:], h1b[:, cch * P:(cch + 1) * P])
                        nc.tensor.matmul(ps3[:], lhsT=hT[:], rhs=w2s[:, cch, :],
                                         start=(cch == 0), stop=(cch == 3))
                    t2 = wp.tile([P, T2COLS], BF, tag="t2")
                    nc.vector.memset(t2[:, 130:T2COLS], 0)
                    nc.vector.tensor_copy(out=t2[:, 0:128], in_=ps3[:, 0:128])
                    nc.vector.tensor_tensor(
                        out=t2[:, 128:130].bitcast(F32), in0=ps3[:, 128:129],
                        in1=afix[:, l, 4:5], op=OP.add)
                    nc.vector.tensor_copy(out=ald2[:, l, :], in_=ps3[:, 129:130])
                    nc.sync.dma_start(ag2_in[l * P:(l + 1) * P, :], t2[:])
                r0, r1 = gi * NPB // NAG, (gi + 1) * NPB // NAG
                nc.gpsimd.collective_compute(
                    "AllGather", OP.bypass, replica_groups=rg,
                    ins=[ag2_in[r0:r1, :].opt()],
                    outs=[ag2_out[gi * NPAD // NAG:
                                  (gi + 1) * NPAD // NAG, :].opt()])

            def _gelu_tanh(xin, out_bf, D):
                x3 = wp.tile([P, D], F32, tag="gx3")
                nc.scalar.activation(x3[:], xin[:], AF.Square)
                nc.vector.tensor_mul(out=x3[:], in0=x3[:], in1=xin[:])
                nc.vector.scalar_tensor_tensor(out=x3[:], in0=x3[:],
                                               scalar=0.044715, in1=xin[:],
                                               op0=OP.mult, op1=OP.add)
                th = wp.tile([P, D], F32, tag="gth")
                nc.scalar.activation(th[:], x3[:], AF.Tanh,
                                     scale=0.797884560802865)
                nc.vector.tensor_scalar_add(out=th[:], in0=th[:], scalar1=1.0)
                nc.vector.tensor_mul(out=th[:], in0=th[:], in1=xin[:])
                nc.scalar.mul(out_bf[:], th[:], 0.5)

            # ---------- batched epilogue: layer 2 + output head ----------
            def epi2(grp, state):
                hbs = [state.pop(f"hb{l}") for l in grp]
                xcs, rstds = [], []
                for hb in hbs:
                    nc.vector.tensor_tensor(out=hb[:], in0=hb[:],
                                            in1=ccb[:, CC_BIAS2:CC_BIAS2 + 128],
                                            op=OP.add)
                    mu = wp.tile([P, 1], F32, tag="e2mu")
                    nc.vector.tensor_reduce(out=mu[:], in_=hb[:],
                                            axis=mybir.AxisListType.X, op=OP.add)
                    nc.scalar.mul(mu[:], mu[:], 1.0 / 128)
                    nc.vector.tensor_scalar_sub(out=hb[:], in0=hb[:], scalar1=mu[:])
                    xcs.append(hb)
                for bi, xc in enumerate(xcs):
                    sq = wp.tile([P, 128], BF, tag="e2sq", name=f"e2sq{bi}")
                    ss = wp.tile([P, 1], F32, tag=f"e2ss{bi}", name=f"e2ss{bi}")
                    nc.scalar.activation(sq[:], xc[:], AF.Square, accum_out=ss[:])
                    rstd = wp.tile([P, 1], F32, tag=f"e2rs{bi}", name=f"e2rs{bi}")
                    rstds.append((ss, rstd))
                for ss, rstd in rstds:
                    nc.scalar.activation(rstd[:], ss[:], AF.Sqrt,
                                         bias=eps_t[:], scale=1.0 / 128)
                for ss, rstd in rstds:
                    nc.vector.reciprocal(rstd[:], rstd[:])
                for xc, (ss, rstd) in zip(xcs, rstds):
                    nc.vector.tensor_scalar_mul(out=xc[:], in0=xc[:],
                                                scalar1=rstd[:])
                    nc.vector.tensor_mul(out=xc[:], in0=xc[:],
                                         in1=ccb[:, CC_G2:CC_G2 + 128])
                    nc.vector.tensor_tensor(out=xc[:], in0=xc[:],
                                            in1=ccb[:, CC_B2:CC_B2 + 128],
                                            op=OP.add)
                h2bs = []
                for bi, xc in enumerate(xcs):
                    h2b = wp.tile([P, 128], BF, tag=f"h2b{bi}", name=f"h2b{bi}")
                    if sim_safe:
                        _gelu_tanh(xc, h2b, 128)
                    else:
                        nc.scalar.activation(h2b[:], xc[:], AF.Gelu)
                    h2bs.append(h2b)
                zs = []
                for l, h2b in zip(grp, h2bs):
                    hoT = wp.tile([P, P], BF, tag="hoT")
                    transpose_to(hoT[:], h2b[:])
                    pso_t = pss.tile([P, 130], F32, tag="mm2")
                    pso = pso_t[:, 0:D_OUT]
                    nc.tensor.matmul(pso[:], lhsT=hoT[:], rhs=wos[:],
                                     start=True, stop=True)
                    z = hp.tile([P, D_OUT], F32, tag=f"z_{len(zs)}")
                    nc.vector.tensor_tensor(out=z[:], in0=pso[:],
                                            in1=ccb[:, CC_BO:CC_BO + 32],
                                            op=OP.add)
                    m = wp.tile([P, 1], F32, tag="zm")
                    nc.vector.tensor_reduce(out=m[:], in_=z[:],
                                            axis=mybir.AxisListType.X, op=OP.max)
                    nc.vector.tensor_scalar_sub(out=z[:], in0=z[:], scalar1=m[:])
                    zs.append((l, z))
                sds = []
                for bi, (l, z) in enumerate(zs):
                    ez = wp.tile([P, D_OUT], F32, tag="ez", name=f"ez{bi}")
                    sden = wp.tile([P, 1], F32, tag=f"sden{bi}", name=f"sden{bi}")
                    nc.scalar.activation(ez[:], z[:], AF.Exp, accum_out=sden[:])
                    sds.append(sden)
                lnds = []
                for bi, sden in enumerate(sds):
                    lnd = wp.tile([P, 1], F32, tag=f"lnd{bi}", name=f"lnd{bi}")
                    nc.scalar.activation(lnd[:], sden[:], AF.Ln)
                    lnds.append(lnd)
                for (l, z), lnd in zip(zs, lnds):
                    res = wp.tile([P, D_OUT], F32, tag="res")
                    nc.vector.tensor_scalar_sub(out=res[:], in0=z[:],
                                                scalar1=lnd[:])
                    nc.sync.dma_start(out[l * P:(l + 1) * P, :], res[:])

            # ---------- emit ----------
            # NOTE: preps capture their gather-source deps at emission time,
            # so every AllGather writing a table must be emitted before the
            # first prep that reads it (layer pipelines run separately).
            phase0()
            groups2 = [[g * GRP + i for i in range(GRP)] for g in grp_order2]
            gat_layer(1, groups2, T1COLS, H1, epi1)
            run_jobs()
            groups4 = [[g * GRP + i for i in range(GRP)] for g in range(NAG)]
            gat_layer(2, groups4, T2COLS, H2, epi2)
            run_jobs()

    nc.compile()
    return nc


_CACHE = {}
_LAST_RUN = {}


def kernel(x, edge_index, g_in, b_in, W1, att1_s, att1_d, bias1, g1, b1,
           W2, att2_s, att2_d, bias2, g2, b2, Wo, bo):
    prep = prepare_inputs(x, edge_index)
    wts = prepare_weights(W1, att1_s, att1_d, bias1, g1, b1, g_in, b_in,
                          W2, att2_s, att2_d, bias2, g2, b2, Wo, bo)

    key = tuple(prep["ghat"])
    if key not in _CACHE:
        _CACHE[key] = build_program(prep["ghat"])
    nc = _CACHE[key]

    in_maps = []
    for c in range(NCORES):
        in_maps.append({
            "x_own": prep["x_own"][c],
            "idxw": prep["idxw"][c],
            "alsfix": prep["alsfix"][c],
            "w1ext": wts["w1ext"],
            "w2ext": wts["w2ext"].astype(BF16),
            "woext": wts["woext"],
            "colconst": wts["colconst"].astype(BF16),
        })

    _LAST_RUN.update(nc=nc, in_maps=in_maps, prep=prep)
    res = bass_utils.run_bass_kernel_spmd(nc, in_maps,
                                          core_ids=list(range(NCORES)))
    outs = [res.results[c]["out"] for c in range(NCORES)]

    newid = prep["newid"]
    blk = newid // P
    core = blk % NCORES
    row = (blk // NCORES) * P + newid % P
    full = np.empty((N, D_OUT), dtype=np.float32)
    for c in range(NCORES):
        sel = core == c
        full[sel] = outs[c][row[sel]]
    return full

